# revision 17
# baseline (speedup 1.0000x reference)
"""Trainium2 Bass kernel for a full transformer block (LN->MHA->LN->FFN).

Sharding: 4-way data-parallel over batch x 2-way tensor-parallel
(heads + FFN hidden split), pairwise AllReduce after attention-proj and FFN.
Core c handles batch c//2 with head-group / hidden-slice c%2.

The program is software-pipelined over t-chunks of 512 rows: phase1 =
LN1+QKV+attention+proj -> per-chunk AllReduce; phase2 = x2+LN2+FFN ->
per-chunk AllReduce -> residual out. Phase2(tc-1) is emitted after
phase1(tc) so collective latency hides under compute (engines run their
instruction streams in order).

LayerNorm scale vectors are folded into Wq/Wk/Wv/W1 rows on the host.

Self-contained: hardcodes the full-problem shapes; builds per-core input
slices on the host, runs one SPMD Bass program on 8 NeuronCores.
"""

import numpy as np
import ml_dtypes

import concourse.bacc as bacc
import concourse.tile as tile
from concourse import mybir
from concourse.bass_utils import run_bass_kernel_spmd

F32 = mybir.dt.float32
F32R = mybir.dt.float32r
BF16 = mybir.dt.bfloat16
EPS = 1e-5


class Cfg:
    def __init__(self, B, T, E, HPC, FH, n_cores):
        self.B, self.T, self.E, self.HPC, self.FH = B, T, E, HPC, FH
        self.n_cores = n_cores
        self.HS = 64
        self.D = HPC * self.HS          # local head dims (= cols of Wq slice)
        self.PAIRS = HPC // 2           # 128-col head-pair groups
        self.TT = T // 128              # t-tiles
        self.QCW = min(512, T)          # q-chunk width for attention
        self.TC = T // self.QCW        # q/t-chunks
        self.KTPQ = self.QCW // 128     # k-tiles per q-chunk block
        self.TPC = self.QCW // 128      # t-tiles per chunk
        self.EC = E // 128              # e-chunks
        self.NH = min(512, E)           # matmul N for E-wide outputs
        self.EH = E // self.NH          # n-halves of E
        self.FC = FH // 128             # FFN hidden chunks
        self.scale = 1.0 / np.sqrt(E)


FULL = Cfg(B=4, T=2048, E=1024, HPC=8, FH=2048, n_cores=8)


def build_nc(cfg):
    c = cfg
    nc = bacc.Bacc(
        "TRN2", target_bir_lowering=False, debug=False, num_devices=c.n_cores
    )
    pairs_rg = [[2 * i, 2 * i + 1] for i in range(c.n_cores // 2)]

    # ---- DRAM I/O ----
    x_d = nc.dram_tensor("x", [c.T, c.E], F32, kind="ExternalInput")
    wq_d = nc.dram_tensor("wq", [c.E, c.D], F32R, kind="ExternalInput")
    wk_d = nc.dram_tensor("wk", [c.E, c.D], F32R, kind="ExternalInput")
    wv_d = nc.dram_tensor("wv", [c.E, c.D], F32R, kind="ExternalInput")
    wo_d = nc.dram_tensor("wo", [c.D, c.E], BF16, kind="ExternalInput")
    w1_d = nc.dram_tensor("w1", [c.E, c.FH], BF16, kind="ExternalInput")
    w2_d = nc.dram_tensor("w2", [c.FH, c.E], BF16, kind="ExternalInput")
    b1_d = nc.dram_tensor("b1", [128, c.FC], F32, kind="ExternalInput")
    bo_d = nc.dram_tensor("bor", [128, c.E], F32, kind="ExternalInput")
    b2_d = nc.dram_tensor("b2r", [128, c.E], F32, kind="ExternalInput")
    msk_d = nc.dram_tensor(
        "masks", [128, c.KTPQ * c.QCW], BF16, kind="ExternalInput"
    )
    id_d = nc.dram_tensor("ident", [128, 128], F32R, kind="ExternalInput")
    out_d = nc.dram_tensor("out", [c.T, c.E], F32, kind="ExternalOutput")

    # ---- persistent SBUF ----
    qkT = nc.alloc_sbuf_tensor("qkT", [128, 2 * c.PAIRS * c.T], BF16).ap()

    def qT(p):
        return qkT[:, p * c.T:(p + 1) * c.T]

    def kT(p):
        return qkT[:, (c.PAIRS + p) * c.T:(c.PAIRS + p + 1) * c.T]

    aT_sb = nc.alloc_sbuf_tensor("aT_sb", [128, c.FC * c.QCW], BF16).ap()

    def aT(f):
        return aT_sb[:, f * c.QCW:(f + 1) * c.QCW]

    v_sb = nc.alloc_sbuf_tensor("v_sb", [128, c.TT * c.HPC * 65], BF16).ap()

    def v_aug(tt, h):
        o = (tt * c.HPC + h) * 65
        return v_sb[:, o:o + 65]

    NSLOT = 6
    attT = nc.alloc_sbuf_tensor("attT", [128, NSLOT * c.QCW], BF16).ap()

    ident = nc.alloc_sbuf_tensor("ident_sb", [128, 128], F32R).ap()
    masks = nc.alloc_sbuf_tensor("masks_sb", [128, c.KTPQ * c.QCW], BF16).ap()
    bo_r = nc.alloc_sbuf_tensor("bo_sb", [128, c.E], F32).ap()
    b2_r = nc.alloc_sbuf_tensor("b2_sb", [128, c.E], F32).ap()
    b1_sb = nc.alloc_sbuf_tensor("b1_sb", [128, c.FC], F32).ap()

    # ---- internal DRAM ----
    ar1_in = nc.dram_tensor("ar1_in", [c.T, c.E], F32, kind="Internal")
    ar1_out = nc.dram_tensor("ar1_out", [c.T, c.E], F32, kind="Internal")
    ar2_in = nc.dram_tensor("ar2_in", [c.T, c.E], F32, kind="Internal")
    ar2_out = nc.dram_tensor("ar2_out", [c.T, c.E], F32, kind="Internal")
    x2_d = nc.dram_tensor("x2_d", [c.T, c.E], F32, kind="Internal")

    with tile.TileContext(nc) as tc:
        with (
            tc.tile_pool(name="io", bufs=3) as io,
            tc.tile_pool(name="hT", bufs=2) as hpool,
            tc.tile_pool(name="yTp", bufs=2) as ypool,
            tc.tile_pool(name="scr", bufs=2) as scr,
            tc.tile_pool(name="stat", bufs=2) as stat,
            tc.tile_pool(name="wqk", bufs=2) as wqk_pool,
            tc.tile_pool(name="w1p", bufs=2) as w1_pool,
            tc.tile_pool(name="w2p", bufs=3) as w2_pool,
            tc.tile_pool(name="wvp", bufs=1) as wv_pool,
            tc.tile_pool(name="wop", bufs=1) as wo_pool,
            tc.tile_pool(name="rcp", bufs=1) as rcp,
            tc.tile_pool(name="ps_sps", bufs=4, space="PSUM") as ps_sps,
            tc.tile_pool(name="ps_tp", bufs=1, space="PSUM") as ps_tp,
            tc.tile_pool(name="ps_acc", bufs=1, space="PSUM") as ps_acc,
            tc.tile_pool(name="ps_yps", bufs=1, space="PSUM") as ps_yps,
        ):
            # ---- consts ----
            nc.gpsimd.dma_start(ident[:], id_d[:])
            nc.gpsimd.dma_start(masks[:], msk_d[:])
            nc.gpsimd.dma_start(bo_r[:], bo_d[:])
            nc.gpsimd.dma_start(b2_r[:], b2_d[:])
            nc.gpsimd.dma_start(b1_sb[:], b1_d[:])

            def layernorm_tile(xt):
                """xt: [128, E] f32 SBUF -> h [128, E] f32r tile.

                Scale weight is pre-folded into the consumer matmul weights.
                rsqrt(v) = exp(-0.5*ln(v)) keeps ACT on one table set
                (natural_log_exp_and_others: exp+ln+relu) for the kernel.
                """
                mu = stat.tile([128, 1], F32, tag="mu")
                nc.vector.reduce_sum(mu[:], xt[:], axis=mybir.AxisListType.X)
                nc.vector.tensor_scalar_mul(mu[:], mu[:], 1.0 / c.E)
                sq = scr.tile([128, c.E], F32, tag="scr_f32")
                nc.vector.tensor_mul(sq[:], xt[:], xt[:])
                ssq = stat.tile([128, 1], F32, tag="ssq")
                nc.vector.reduce_sum(ssq[:], sq[:], axis=mybir.AxisListType.X)
                musq = stat.tile([128, 1], F32, tag="musq")
                nc.vector.tensor_mul(musq[:], mu[:], mu[:])
                varp = stat.tile([128, 1], F32, tag="varp")
                nc.vector.tensor_scalar(
                    varp[:], ssq[:], 1.0 / c.E, None, mybir.AluOpType.mult
                )
                nc.vector.tensor_scalar(
                    varp[:], varp[:], musq[:], EPS,
                    mybir.AluOpType.subtract, mybir.AluOpType.add,
                )
                lnv = stat.tile([128, 1], F32, tag="lnv")
                nc.scalar.activation(
                    lnv[:], varp[:], mybir.ActivationFunctionType.Ln
                )
                rsig = stat.tile([128, 1], F32, tag="rsig")
                nc.scalar.activation(
                    rsig[:], lnv[:], mybir.ActivationFunctionType.Exp,
                    scale=-0.5,
                )
                h = scr.tile([128, c.E], F32R, tag="h")
                nc.vector.tensor_scalar(
                    h[:], xt[:], mu[:], rsig[:],
                    mybir.AluOpType.subtract, mybir.AluOpType.mult,
                )
                return h

            TG = 4  # transposes per psum tile

            def transpose_to(h, hTc, tt_loc):
                """h [128,E] f32r -> hTc e-chunk columns tt_loc (transposed)."""
                dst3 = hTc.rearrange("p (e w) -> p e w", e=c.EC)[
                    :, :, tt_loc * 128:(tt_loc + 1) * 128
                ]
                for g0 in range(0, c.EC, TG):
                    tp = ps_tp.tile([128, TG * 128], F32R, tag="tp")
                    for i in range(TG):
                        e = g0 + i
                        nc.tensor.matmul(
                            tp[:, i * 128:(i + 1) * 128],
                            h[:, e * 128:(e + 1) * 128],
                            ident[:],
                            is_transpose=True, start=True, stop=True,
                        )
                    nc.vector.tensor_copy(
                        dst3[:, g0:g0 + TG, :],
                        tp[:].rearrange("p (g w) -> p g w", g=TG),
                    )

            slot_ctr = [0]

            def att_block(p, qc, yTc):
                """Attention for head pair p, q-chunk qc (kT/v ready)."""
                last = c.KTPQ * qc + c.KTPQ - 1
                q0 = qc * c.QCW
                yps_e = ps_yps.tile([65, c.QCW], F32, tag="yps_e")
                yps_o = ps_yps.tile([65, c.QCW], F32, tag="yps_o")
                pend = []

                def issue_av(kt, a_e, a_o):
                    st, sp = kt == 0, kt == last
                    nc.tensor.matmul(
                        yps_e[:], v_aug(kt, 2 * p), a_e, start=st, stop=sp
                    )
                    nc.tensor.matmul(
                        yps_o[:], v_aug(kt, 2 * p + 1), a_o, start=st, stop=sp
                    )

                for kt in range(last + 1):
                    sps_e = ps_sps.tile([128, c.QCW], F32, tag="sps")
                    sps_o = ps_sps.tile([128, c.QCW], F32, tag="sps")
                    for hh, sps in ((0, sps_e), (1, sps_o)):
                        off = hh * 64
                        nc.tensor.matmul(
                            sps[:],
                            kT(p)[off:off + 64, kt * 128:(kt + 1) * 128],
                            qT(p)[off:off + 64, q0:q0 + c.QCW],
                            start=True, stop=True,
                            tile_position=(off, 0),
                        )
                    s0 = (slot_ctr[0] % NSLOT) * c.QCW
                    a_e = attT[:, s0:s0 + c.QCW]
                    slot_ctr[0] += 1
                    s1 = (slot_ctr[0] % NSLOT) * c.QCW
                    a_o = attT[:, s1:s1 + c.QCW]
                    slot_ctr[0] += 1
                    nc.scalar.activation(
                        a_e, sps_e[:], mybir.ActivationFunctionType.Exp
                    )
                    nc.scalar.activation(
                        a_o, sps_o[:], mybir.ActivationFunctionType.Exp
                    )
                    j = kt - c.KTPQ * qc
                    if j >= 0:  # diagonal block: causal mask
                        m = masks[:, j * c.QCW:(j + 1) * c.QCW]
                        nc.vector.tensor_mul(a_e, a_e, m)
                        nc.vector.tensor_mul(a_o, a_o, m)
                    pend.append((kt, a_e, a_o))
                    if len(pend) > 1:
                        issue_av(*pend.pop(0))
                while pend:
                    issue_av(*pend.pop(0))

                # normalize: yTc[p] rows = yps[0:64] * (1/yps[64])
                for hh, yps in ((0, yps_e), (1, yps_o)):
                    rc = rcp.tile([1, c.QCW], F32, tag="rc")
                    nc.vector.reciprocal(rc[:], yps[64:65, :])
                    rb = rcp.tile([64, c.QCW], F32, tag="rb")
                    nc.gpsimd.partition_broadcast(rb[:], rc[:])
                    nc.vector.tensor_mul(
                        yTc[hh * 64:hh * 64 + 64, p * c.QCW:(p + 1) * c.QCW],
                        rb[:], yps[0:64, :],
                    )

            def phase1a(tcc):
                """LN1 + V + QK for chunk tcc; returns wot for phase1b."""
                r0 = tcc * c.QCW  # first row of chunk
                hTc = hpool.tile([128, c.EC * c.QCW], F32R, tag="hT")

                # x tiles first so LN can start before weight DMAs queue
                xts = []
                for tt_loc in range(c.TPC):
                    tt = tcc * c.TPC + tt_loc
                    xt = io.tile([128, c.E], F32, tag="io")
                    nc.sync.dma_start(xt[:], x_d[tt * 128:(tt + 1) * 128, :])
                    xts.append(xt)

                # weight prefetch for this chunk
                wvt = wv_pool.tile([128, c.EC * c.D], F32R, tag="wv")
                nc.sync.dma_start(
                    wvt[:].rearrange("p (e d) -> p e d", e=c.EC),
                    wv_d[:].rearrange("(e p) d -> p e d", p=128),
                )
                wot = wo_pool.tile([128, c.PAIRS * c.E], BF16, tag="wo")
                nc.sync.dma_start(
                    wot[:].rearrange("p (d e) -> p d e", d=c.PAIRS),
                    wo_d[:].rearrange("(d p) e -> p d e", p=128),
                )

                for tt_loc in range(c.TPC):
                    h = layernorm_tile(xts[tt_loc])
                    transpose_to(h, hTc[:, :], tt_loc)

                # V for this chunk's t-tiles
                for tt_loc in range(c.TPC):
                    tt = tcc * c.TPC + tt_loc
                    vps = ps_acc.tile([128, c.D], F32, tag="acc")
                    for e in range(c.EC):
                        nc.tensor.matmul(
                            vps[:],
                            hTc[:, e * c.QCW + tt_loc * 128:][:, :128],
                            wvt[:, e * c.D:(e + 1) * c.D],
                            start=(e == 0), stop=(e == c.EC - 1),
                        )
                    for h_i in range(c.HPC):
                        nc.vector.tensor_copy(
                            v_aug(tt, h_i)[:, 0:64],
                            vps[:, h_i * 64:(h_i + 1) * 64],
                        )
                        nc.vector.memset(v_aug(tt, h_i)[:, 64:65], 1.0)

                # Q/K for this chunk
                for p in range(c.PAIRS):
                    wqt = wqk_pool.tile([128, c.E], F32R, tag="wqk")
                    nc.sync.dma_start(
                        wqt[:].rearrange("p (e m) -> p e m", e=c.EC),
                        wq_d[:, p * 128:(p + 1) * 128].rearrange(
                            "(e p) m -> p e m", p=128
                        ),
                    )
                    wkt = wqk_pool.tile([128, c.E], F32R, tag="wqk")
                    nc.sync.dma_start(
                        wkt[:].rearrange("p (e m) -> p e m", e=c.EC),
                        wk_d[:, p * 128:(p + 1) * 128].rearrange(
                            "(e p) m -> p e m", p=128
                        ),
                    )
                    for dst, wt, do_scale in (
                        (qT, wqt, True), (kT, wkt, False)
                    ):
                        pp = ps_sps.tile([128, c.QCW], F32, tag="sps")
                        for e in range(c.EC):
                            nc.tensor.matmul(
                                pp[:], wt[:, e * 128:(e + 1) * 128],
                                hTc[:, e * c.QCW:(e + 1) * c.QCW],
                                start=(e == 0), stop=(e == c.EC - 1),
                            )
                        d = dst(p)[:, r0:r0 + c.QCW]
                        if do_scale:
                            nc.vector.tensor_scalar_mul(d, pp[:], c.scale)
                        else:
                            nc.vector.tensor_copy(d, pp[:])

                return wot

            def phase1b(tcc, wot):
                """Attention + proj -> ar1_in chunk + AllReduce."""
                r0 = tcc * c.QCW
                yTc = ypool.tile([128, c.PAIRS * c.QCW], BF16, tag="yT")
                for p in range(c.PAIRS):
                    att_block(p, tcc, yTc[:, :])

                for tt_loc in range(c.TPC):
                    tt = tcc * c.TPC + tt_loc
                    pt = io.tile([128, c.E], F32, tag="io")
                    for eh in range(c.EH):
                        pp = ps_sps.tile([128, c.NH], F32, tag="sps")
                        for d in range(c.PAIRS):
                            nc.tensor.matmul(
                                pp[:],
                                yTc[:, d * c.QCW + tt_loc * 128:][:, :128],
                                wot[:, d * c.E + eh * c.NH:][:, :c.NH],
                                start=(d == 0), stop=(d == c.PAIRS - 1),
                            )
                        nc.vector.tensor_copy(
                            pt[:, eh * c.NH:(eh + 1) * c.NH], pp[:]
                        )
                    nc.sync.dma_start(ar1_in[tt * 128:(tt + 1) * 128, :], pt[:])

                rows = slice(r0, r0 + c.QCW)
                if c.n_cores == 1:  # timeline/profiling variant
                    nc.sync.dma_start(ar1_out[rows, :], ar1_in[rows, :])
                else:
                    nc.gpsimd.collective_compute(
                        "AllReduce", mybir.AluOpType.add,
                        replica_groups=pairs_rg,
                        ins=[ar1_in[rows, :]], outs=[ar1_out[rows, :]],
                    )

            def phase2a(tcc):
                """x2 + LN2 -> transposed h2 chunk (bf16); returns it."""
                hTc = hpool.tile([128, c.EC * c.QCW], BF16, tag="hT")

                for tt_loc in range(c.TPC):
                    tt = tcc * c.TPC + tt_loc
                    xt = io.tile([128, c.E], F32, tag="io")
                    nc.sync.dma_start(xt[:], x_d[tt * 128:(tt + 1) * 128, :])
                    at = io.tile([128, c.E], F32, tag="io")
                    nc.sync.dma_start(
                        at[:], ar1_out[tt * 128:(tt + 1) * 128, :]
                    )
                    x2 = io.tile([128, c.E], F32, tag="io")
                    nc.gpsimd.tensor_add(x2[:], xt[:], at[:])
                    nc.gpsimd.tensor_add(x2[:], x2[:], bo_r[:])
                    nc.sync.dma_start(x2_d[tt * 128:(tt + 1) * 128, :], x2[:])
                    h2 = layernorm_tile(x2)
                    transpose_to(h2, hTc[:, :], tt_loc)
                return hTc

            def phase2b(tcc, hTc):
                """FFN -> ar2_in chunk + AllReduce."""
                # prefetch first ffn2 weight tile under the ffn1 stretch
                w2t0 = w2_pool.tile([128, 2 * c.NH], BF16, tag="w2")
                nc.sync.dma_start(
                    w2t0[:].rearrange("p (g n) -> p g n", g=2),
                    w2_d[0:256, 0:c.NH].rearrange("(g p) n -> p g n", p=128),
                )
                # FFN layer 1: aT[f] = relu(w1_f.T @ h2T + b1_f)
                for f in range(c.FC):
                    w1t = w1_pool.tile([128, c.E], BF16, tag="w1")
                    nc.sync.dma_start(
                        w1t[:].rearrange("p (e m) -> p e m", e=c.EC),
                        w1_d[:, f * 128:(f + 1) * 128].rearrange(
                            "(e p) m -> p e m", p=128
                        ),
                    )
                    ap_ = ps_acc.tile([128, c.QCW], F32, tag="acc")
                    for e in range(c.EC):
                        nc.tensor.matmul(
                            ap_[:], w1t[:, e * 128:(e + 1) * 128],
                            hTc[:, e * c.QCW:(e + 1) * c.QCW],
                            start=(e == 0), stop=(e == c.EC - 1),
                        )
                    nc.scalar.activation(
                        aT(f), ap_[:], mybir.ActivationFunctionType.Relu,
                        bias=b1_sb[:, f:f + 1],
                    )

                # FFN layer 2: ff[t, e] = sum_f aT[f].T @ w2_f
                # first w2 tile prefetched during ffn1 (see above)
                for eh in range(c.EH):
                    ffps = []
                    for _fi in range(c.TPC):
                        fftile = ps_sps.tile([128, c.NH], F32, tag="sps")
                        ffps.append(fftile)
                    for fg in range(c.FC // 2):
                        if eh == 0 and fg == 0:
                            w2t = w2t0
                        else:
                            w2t = w2_pool.tile([128, 2 * c.NH], BF16, tag="w2")
                            nc.sync.dma_start(
                                w2t[:].rearrange("p (g n) -> p g n", g=2),
                                w2_d[
                                    2 * fg * 128:(2 * fg + 2) * 128,
                                    eh * c.NH:(eh + 1) * c.NH,
                                ].rearrange("(g p) n -> p g n", p=128),
                            )
                        for gi in range(2):
                            f = 2 * fg + gi
                            for tt_loc in range(c.TPC):
                                nc.tensor.matmul(
                                    ffps[tt_loc][:],
                                    aT(f)[:, tt_loc * 128:(tt_loc + 1) * 128],
                                    w2t[:, gi * c.NH:(gi + 1) * c.NH],
                                    start=(f == 0), stop=(f == c.FC - 1),
                                )
                    for tt_loc in range(c.TPC):
                        tt = tcc * c.TPC + tt_loc
                        ft = io.tile([128, c.NH], F32, tag="ffout")
                        nc.vector.tensor_copy(ft[:], ffps[tt_loc][:])
                        nc.sync.dma_start(
                            ar2_in[tt * 128:(tt + 1) * 128, eh * c.NH:][
                                :, :c.NH
                            ],
                            ft[:],
                        )

                rows = slice(tcc * c.QCW, (tcc + 1) * c.QCW)
                if c.n_cores == 1:
                    nc.sync.dma_start(ar2_out[rows, :], ar2_in[rows, :])
                else:
                    nc.gpsimd.collective_compute(
                        "AllReduce", mybir.AluOpType.add,
                        replica_groups=pairs_rg,
                        ins=[ar2_in[rows, :]], outs=[ar2_out[rows, :]],
                    )

            def final(tcc):
                """out chunk = x2 + ff_sum + b2."""
                for tt_loc in range(c.TPC):
                    tt = tcc * c.TPC + tt_loc
                    x2 = io.tile([128, c.E], F32, tag="io")
                    nc.sync.dma_start(x2[:], x2_d[tt * 128:(tt + 1) * 128, :])
                    ft = io.tile([128, c.E], F32, tag="io")
                    nc.sync.dma_start(
                        ft[:], ar2_out[tt * 128:(tt + 1) * 128, :]
                    )
                    ot = io.tile([128, c.E], F32, tag="io")
                    nc.gpsimd.tensor_add(ot[:], x2[:], ft[:])
                    nc.gpsimd.tensor_add(ot[:], ot[:], b2_r[:])
                    nc.sync.dma_start(out_d[tt * 128:(tt + 1) * 128, :], ot[:])

            # ---- software-pipelined emission over chunks ----
            h2prev = None
            for tcc in range(c.TC):
                wot = phase1a(tcc)
                if tcc >= 1:
                    h2prev = phase2a(tcc - 1)
                phase1b(tcc, wot)
                if tcc >= 2:
                    final(tcc - 2)
                if tcc >= 1:
                    phase2b(tcc - 1, h2prev)
            h2prev = phase2a(c.TC - 1)
            phase2b(c.TC - 1, h2prev)
            for tcc in range(max(0, c.TC - 2), c.TC):
                final(tcc)

    nc.compile()
    return nc


def make_masks(cfg):
    c = cfg
    m = np.zeros((128, c.KTPQ * c.QCW), dtype=np.float32)
    for j in range(c.KTPQ):
        k = np.arange(128)[:, None]
        q = np.arange(c.QCW)[None, :]
        m[:, j * c.QCW:(j + 1) * c.QCW] = (j * 128 + k <= q).astype(np.float32)
    return np.ascontiguousarray(m.astype(ml_dtypes.bfloat16))


def make_in_maps(cfg, inputs):
    """Build the per-core input dicts from the full problem inputs."""
    c = cfg
    x = np.asarray(inputs["x"], dtype=np.float32)
    ln1 = np.asarray(inputs["ln1_w"], dtype=np.float32)
    ln2 = np.asarray(inputs["ln2_w"], dtype=np.float32)
    # fold LN scale vectors into the consumer weight rows
    Wq = ln1[:, None] * np.asarray(inputs["Wq"], dtype=np.float32)
    Wk = ln1[:, None] * np.asarray(inputs["Wk"], dtype=np.float32)
    Wv = ln1[:, None] * np.asarray(inputs["Wv"], dtype=np.float32)
    W1 = ln2[:, None] * np.asarray(inputs["W1"], dtype=np.float32)
    Wo = np.asarray(inputs["Wo"], dtype=np.float32)
    W2 = np.asarray(inputs["W2"], dtype=np.float32)
    bo = np.asarray(inputs["bo"], dtype=np.float32)
    b1 = np.asarray(inputs["b1"], dtype=np.float32)
    b2 = np.asarray(inputs["b2"], dtype=np.float32)

    def rep(v):
        return np.ascontiguousarray(
            np.broadcast_to(v[None, :], (128, c.E)).astype(np.float32)
        )

    consts = {
        "bor": rep(bo), "b2r": rep(b2),
        "masks": make_masks(c),
        "ident": np.eye(128, dtype=np.float32),
    }
    in_maps = []
    for core in range(c.n_cores):
        b, g = core // 2, core % 2
        d0, d1 = g * c.D, (g + 1) * c.D
        f0, f1 = g * c.FH, (g + 1) * c.FH
        m = {
            "x": np.ascontiguousarray(x[b]),
            "wq": np.ascontiguousarray(Wq[:, d0:d1]),
            "wk": np.ascontiguousarray(Wk[:, d0:d1]),
            "wv": np.ascontiguousarray(Wv[:, d0:d1]),
            "wo": np.ascontiguousarray(Wo[d0:d1, :].astype(ml_dtypes.bfloat16)),
            "w1": np.ascontiguousarray(W1[:, f0:f1].astype(ml_dtypes.bfloat16)),
            "w2": np.ascontiguousarray(W2[f0:f1, :].astype(ml_dtypes.bfloat16)),
            "b1": np.ascontiguousarray(b1[f0:f1].reshape(c.FC, 128).T),
        }
        m.update(consts)
        in_maps.append(m)
    return in_maps


_NC_CACHE = {}


def get_nc(cfg):
    key = (cfg.B, cfg.T, cfg.E, cfg.HPC, cfg.FH, cfg.n_cores)
    if key not in _NC_CACHE:
        _NC_CACHE[key] = build_nc(cfg)
    return _NC_CACHE[key]


def kernel(**inputs) -> np.ndarray:
    c = FULL
    nc = get_nc(c)
    in_maps = make_in_maps(c, inputs)
    res = run_bass_kernel_spmd(nc, in_maps, core_ids=list(range(c.n_cores)))
    out = np.stack([res.results[2 * b]["out"] for b in range(c.B)], axis=0)
    return out.astype(np.float32)


# revision 21
# speedup vs baseline: 36.0432x; 36.0432x over previous
"""Trainium2 Bass kernel for a full transformer block (LN->MHA->LN->FFN).

Sharding: 4-way data-parallel over batch x 2-way tensor-parallel
(heads + FFN hidden split), pairwise AllReduce after attention-proj and FFN.
Core c handles batch c//2 with head-group / hidden-slice c%2.

The program is software-pipelined over t-chunks of 512 rows: phase1 =
LN1+QKV+attention+proj -> per-chunk AllReduce; phase2 = x2+LN2+FFN ->
per-chunk AllReduce -> residual out. Phase2(tc-1) is emitted after
phase1(tc) so collective latency hides under compute (engines run their
instruction streams in order).

LayerNorm scale vectors are folded into Wq/Wk/Wv/W1 rows on the host.

Self-contained: hardcodes the full-problem shapes; builds per-core input
slices on the host, runs one SPMD Bass program on 8 NeuronCores.
"""

import numpy as np
import ml_dtypes

import concourse.bacc as bacc
import concourse.tile as tile
from concourse import mybir
from concourse.bass_utils import run_bass_kernel_spmd

F32 = mybir.dt.float32
F32R = mybir.dt.float32r
BF16 = mybir.dt.bfloat16
EPS = 1e-5


class Cfg:
    def __init__(self, B, T, E, HPC, FH, n_cores):
        self.B, self.T, self.E, self.HPC, self.FH = B, T, E, HPC, FH
        self.n_cores = n_cores
        self.HS = 64
        self.D = HPC * self.HS          # local head dims (= cols of Wq slice)
        self.PAIRS = HPC // 2           # 128-col head-pair groups
        self.TT = T // 128              # t-tiles
        self.QCW = min(512, T)          # q-chunk width for attention
        self.TC = T // self.QCW        # q/t-chunks
        self.KTPQ = self.QCW // 128     # k-tiles per q-chunk block
        self.TPC = self.QCW // 128      # t-tiles per chunk
        self.EC = E // 128              # e-chunks
        self.NH = min(512, E)           # matmul N for E-wide outputs
        self.EH = E // self.NH          # n-halves of E
        self.FC = FH // 128             # FFN hidden chunks
        self.scale = 1.0 / np.sqrt(E)


FULL = Cfg(B=4, T=2048, E=1024, HPC=8, FH=2048, n_cores=8)


def build_nc(cfg):
    c = cfg
    nc = bacc.Bacc(
        "TRN2", target_bir_lowering=False, debug=False, num_devices=c.n_cores
    )
    pairs_rg = [[2 * i, 2 * i + 1] for i in range(c.n_cores // 2)]

    # ---- DRAM I/O ----
    x_d = nc.dram_tensor("x", [c.T, c.E], F32, kind="ExternalInput")
    wq_d = nc.dram_tensor("wq", [c.E, c.D], BF16, kind="ExternalInput")
    wk_d = nc.dram_tensor("wk", [c.E, c.D], BF16, kind="ExternalInput")
    wv_d = nc.dram_tensor("wv", [c.E, c.D], BF16, kind="ExternalInput")
    wo_d = nc.dram_tensor("wo", [c.D, c.E], BF16, kind="ExternalInput")
    w1_d = nc.dram_tensor("w1", [c.E, c.FH], BF16, kind="ExternalInput")
    w2_d = nc.dram_tensor("w2", [c.FH, c.E], BF16, kind="ExternalInput")
    b1_d = nc.dram_tensor("b1", [128, c.FC], F32, kind="ExternalInput")
    bo_d = nc.dram_tensor("bor", [128, c.E], F32, kind="ExternalInput")
    b2_d = nc.dram_tensor("b2r", [128, c.E], F32, kind="ExternalInput")
    msk_d = nc.dram_tensor(
        "masks", [128, c.KTPQ * c.QCW], BF16, kind="ExternalInput"
    )
    id_d = nc.dram_tensor("ident", [128, 128], F32R, kind="ExternalInput")
    out_d = nc.dram_tensor("out", [c.T, c.E], F32, kind="ExternalOutput")

    # ---- persistent SBUF ----
    qkT = nc.alloc_sbuf_tensor("qkT", [128, 2 * c.PAIRS * c.T], BF16).ap()

    def qT(p):
        return qkT[:, p * c.T:(p + 1) * c.T]

    def kT(p):
        return qkT[:, (c.PAIRS + p) * c.T:(c.PAIRS + p + 1) * c.T]

    aT_sb = nc.alloc_sbuf_tensor("aT_sb", [128, c.FC * c.QCW], BF16).ap()

    def aT(f):
        return aT_sb[:, f * c.QCW:(f + 1) * c.QCW]

    v_sb = nc.alloc_sbuf_tensor("v_sb", [128, c.TT * c.HPC * 65], BF16).ap()

    def v_aug(tt, h):
        o = (tt * c.HPC + h) * 65
        return v_sb[:, o:o + 65]

    NSLOT = 6
    attT = nc.alloc_sbuf_tensor("attT", [128, NSLOT * c.QCW], BF16).ap()

    ident = nc.alloc_sbuf_tensor("ident_sb", [128, 128], F32R).ap()
    masks = nc.alloc_sbuf_tensor("masks_sb", [128, c.KTPQ * c.QCW], BF16).ap()
    bo_r = nc.alloc_sbuf_tensor("bo_sb", [128, c.E], F32).ap()
    bob2 = nc.alloc_sbuf_tensor("bob2_sb", [128, c.E], F32).ap()
    b1_sb = nc.alloc_sbuf_tensor("b1_sb", [128, c.FC], F32).ap()

    # ---- internal DRAM ----
    ar1_in = nc.dram_tensor("ar1_in", [c.T, c.E], F32, kind="Internal")
    ar1_out = nc.dram_tensor("ar1_out", [c.T, c.E], F32, kind="Internal")
    ar2_in = nc.dram_tensor("ar2_in", [c.T, c.E], F32, kind="Internal")
    ar2_out = nc.dram_tensor("ar2_out", [c.T, c.E], F32, kind="Internal")

    with tile.TileContext(nc) as tc:
        with (
            tc.tile_pool(name="io", bufs=4) as io,
            tc.tile_pool(name="hT", bufs=2) as hpool,
            tc.tile_pool(name="yTp", bufs=2) as ypool,
            tc.tile_pool(name="scr", bufs=2) as scr,
            tc.tile_pool(name="stat", bufs=2) as stat,
            tc.tile_pool(name="wqk", bufs=3) as wqk_pool,
            tc.tile_pool(name="w1p", bufs=2) as w1_pool,
            tc.tile_pool(name="w2p", bufs=3) as w2_pool,
            tc.tile_pool(name="wvp", bufs=1) as wv_pool,
            tc.tile_pool(name="wop", bufs=1) as wo_pool,
            tc.tile_pool(name="rcp", bufs=1) as rcp,
            tc.tile_pool(name="ps_sps", bufs=4, space="PSUM") as ps_sps,
            tc.tile_pool(name="ps_tp", bufs=1, space="PSUM") as ps_tp,
            tc.tile_pool(name="ps_acc", bufs=1, space="PSUM") as ps_acc,
            tc.tile_pool(name="ps_yps", bufs=1, space="PSUM") as ps_yps,
        ):
            # ---- consts ----
            nc.gpsimd.dma_start(ident[:], id_d[:])
            nc.gpsimd.dma_start(masks[:], msk_d[:])
            nc.gpsimd.dma_start(bo_r[:], bo_d[:])
            nc.gpsimd.dma_start(bob2[:], b2_d[:])
            nc.gpsimd.dma_start(b1_sb[:], b1_d[:])

            def layernorm_tile(xt):
                """xt: [128, E] f32 SBUF -> h [128, E] f32r tile.

                Scale weight is pre-folded into the consumer matmul weights.
                rsqrt(v) = exp(-0.5*ln(v)) keeps ACT on one table set
                (natural_log_exp_and_others: exp+ln+relu) for the kernel.
                """
                ng = c.E // 512
                bst = stat.tile([128, 6 * ng], F32, tag="bst")
                bst3 = bst[:].rearrange("p (g s) -> p g s", g=ng)
                for g in range(ng):
                    nc.vector.bn_stats(
                        bst3[:, g:g + 1, :],
                        xt[:, g * 512:(g + 1) * 512].rearrange(
                            "p (g w) -> p g w", g=1
                        ),
                    )
                mv = stat.tile([128, 2], F32, tag="mv")
                nc.vector.bn_aggr(
                    mv[:], bst[:].rearrange("p (g s) -> p g s", g=ng)
                )
                mu = mv[:, 0:1]
                varp = stat.tile([128, 1], F32, tag="varp")
                nc.vector.tensor_scalar_add(varp[:], mv[:, 1:2], EPS)
                lnv = stat.tile([128, 1], F32, tag="lnv")
                nc.scalar.activation(
                    lnv[:], varp[:], mybir.ActivationFunctionType.Ln
                )
                rsig = stat.tile([128, 1], F32, tag="rsig")
                nc.scalar.activation(
                    rsig[:], lnv[:], mybir.ActivationFunctionType.Exp,
                    scale=-0.5,
                )
                h = scr.tile([128, c.E], F32R, tag="h")
                nc.vector.tensor_scalar(
                    h[:], xt[:], mu, rsig[:],
                    mybir.AluOpType.subtract, mybir.AluOpType.mult,
                )
                return h

            TG = 4  # transposes per psum tile

            def transpose_to(h, hTc, tt_loc):
                """h [128,E] f32r -> hTc e-chunk columns tt_loc (transposed)."""
                dst3 = hTc.rearrange("p (e w) -> p e w", e=c.EC)[
                    :, :, tt_loc * 128:(tt_loc + 1) * 128
                ]
                for g0 in range(0, c.EC, TG):
                    tp = ps_tp.tile([128, TG * 128], F32R, tag="tp")
                    for i in range(TG):
                        e = g0 + i
                        nc.tensor.matmul(
                            tp[:, i * 128:(i + 1) * 128],
                            h[:, e * 128:(e + 1) * 128],
                            ident[:],
                            is_transpose=True, start=True, stop=True,
                        )
                    nc.vector.tensor_copy(
                        dst3[:, g0:g0 + TG, :],
                        tp[:].rearrange("p (g w) -> p g w", g=TG),
                    )

            slot_ctr = [0]

            def att_block(p, qc, yTc):
                """Attention for head pair p, q-chunk qc (kT/v ready)."""
                last = c.KTPQ * qc + c.KTPQ - 1
                q0 = qc * c.QCW
                yps_e = ps_yps.tile([65, c.QCW], F32, tag="yps_e")
                yps_o = ps_yps.tile([65, c.QCW], F32, tag="yps_o")
                pend = []

                def issue_av(kt, a_e, a_o):
                    st, sp = kt == 0, kt == last
                    nc.tensor.matmul(
                        yps_e[:], v_aug(kt, 2 * p), a_e, start=st, stop=sp
                    )
                    nc.tensor.matmul(
                        yps_o[:], v_aug(kt, 2 * p + 1), a_o, start=st, stop=sp
                    )

                for kt in range(last + 1):
                    sps_e = ps_sps.tile([128, c.QCW], F32, tag="sps")
                    sps_o = ps_sps.tile([128, c.QCW], F32, tag="sps")
                    for hh, sps in ((0, sps_e), (1, sps_o)):
                        off = hh * 64
                        nc.tensor.matmul(
                            sps[:],
                            kT(p)[off:off + 64, kt * 128:(kt + 1) * 128],
                            qT(p)[off:off + 64, q0:q0 + c.QCW],
                            start=True, stop=True,
                            tile_position=(off, 0),
                        )
                    s0 = (slot_ctr[0] % NSLOT) * c.QCW
                    a_e = attT[:, s0:s0 + c.QCW]
                    slot_ctr[0] += 1
                    s1 = (slot_ctr[0] % NSLOT) * c.QCW
                    a_o = attT[:, s1:s1 + c.QCW]
                    slot_ctr[0] += 1
                    nc.scalar.activation(
                        a_e, sps_e[:], mybir.ActivationFunctionType.Exp
                    )
                    nc.scalar.activation(
                        a_o, sps_o[:], mybir.ActivationFunctionType.Exp
                    )
                    j = kt - c.KTPQ * qc
                    if j >= 0:  # diagonal block: causal mask
                        m = masks[:, j * c.QCW:(j + 1) * c.QCW]
                        nc.vector.tensor_mul(a_e, a_e, m)
                        nc.vector.tensor_mul(a_o, a_o, m)
                    pend.append((kt, a_e, a_o))
                    if len(pend) > 1:
                        issue_av(*pend.pop(0))
                while pend:
                    issue_av(*pend.pop(0))

                # normalize: yTc[p] rows = yps[0:64] * (1/yps[64])
                for hh, yps in ((0, yps_e), (1, yps_o)):
                    rc = rcp.tile([1, c.QCW], F32, tag="rc")
                    nc.vector.reciprocal(rc[:], yps[64:65, :])
                    rb = rcp.tile([64, c.QCW], F32, tag="rb")
                    nc.gpsimd.partition_broadcast(rb[:], rc[:])
                    nc.vector.tensor_mul(
                        yTc[hh * 64:hh * 64 + 64, p * c.QCW:(p + 1) * c.QCW],
                        rb[:], yps[0:64, :],
                    )

            def phase1a(tcc):
                """LN1 + V + QK for chunk tcc; returns wot for phase1b."""
                r0 = tcc * c.QCW  # first row of chunk
                hTc = hpool.tile([128, c.EC * c.QCW], BF16, tag="hT")

                # x tiles first so LN can start before weight DMAs queue
                xts = []
                for tt_loc in range(c.TPC):
                    tt = tcc * c.TPC + tt_loc
                    xt = io.tile([128, c.E], F32, tag="io")
                    nc.sync.dma_start(xt[:], x_d[tt * 128:(tt + 1) * 128, :])
                    xts.append(xt)

                # weight prefetch for this chunk
                wvt = wv_pool.tile([128, c.EC * c.D], BF16, tag="wv")
                nc.sync.dma_start(
                    wvt[:].rearrange("p (e d) -> p e d", e=c.EC),
                    wv_d[:].rearrange("(e p) d -> p e d", p=128),
                )
                wot = wo_pool.tile([128, c.PAIRS * c.E], BF16, tag="wo")
                nc.sync.dma_start(
                    wot[:].rearrange("p (d e) -> p d e", d=c.PAIRS),
                    wo_d[:].rearrange("(d p) e -> p d e", p=128),
                )

                for tt_loc in range(c.TPC):
                    h = layernorm_tile(xts[tt_loc])
                    transpose_to(h, hTc[:, :], tt_loc)

                # V for this chunk's t-tiles
                for tt_loc in range(c.TPC):
                    tt = tcc * c.TPC + tt_loc
                    vps = ps_acc.tile([128, c.D], F32, tag="acc")
                    for e in range(c.EC):
                        nc.tensor.matmul(
                            vps[:],
                            hTc[:, e * c.QCW + tt_loc * 128:][:, :128],
                            wvt[:, e * c.D:(e + 1) * c.D],
                            start=(e == 0), stop=(e == c.EC - 1),
                        )
                    vdst = v_sb[
                        :, tt * c.HPC * 65:(tt + 1) * c.HPC * 65
                    ].rearrange("p (h w) -> p h w", w=65)
                    nc.vector.tensor_copy(
                        vdst[:, :, 0:64],
                        vps[:].rearrange("p (h w) -> p h w", w=64),
                    )
                    nc.vector.memset(vdst[:, :, 64:65], 1.0)

                # Q/K for this chunk
                for p in range(c.PAIRS):
                    wqt = wqk_pool.tile([128, c.E], BF16, tag="wqk")
                    nc.sync.dma_start(
                        wqt[:].rearrange("p (e m) -> p e m", e=c.EC),
                        wq_d[:, p * 128:(p + 1) * 128].rearrange(
                            "(e p) m -> p e m", p=128
                        ),
                    )
                    wkt = wqk_pool.tile([128, c.E], BF16, tag="wqk")
                    nc.sync.dma_start(
                        wkt[:].rearrange("p (e m) -> p e m", e=c.EC),
                        wk_d[:, p * 128:(p + 1) * 128].rearrange(
                            "(e p) m -> p e m", p=128
                        ),
                    )
                    for dst, wt, do_scale in (
                        (qT, wqt, True), (kT, wkt, False)
                    ):
                        pp = ps_sps.tile([128, c.QCW], F32, tag="sps")
                        for e in range(c.EC):
                            nc.tensor.matmul(
                                pp[:], wt[:, e * 128:(e + 1) * 128],
                                hTc[:, e * c.QCW:(e + 1) * c.QCW],
                                start=(e == 0), stop=(e == c.EC - 1),
                            )
                        d = dst(p)[:, r0:r0 + c.QCW]
                        if do_scale:
                            nc.vector.tensor_scalar_mul(d, pp[:], c.scale)
                        else:
                            nc.vector.tensor_copy(d, pp[:])

                return wot

            def phase1b(tcc, wot):
                """Attention + proj -> ar1_in chunk + AllReduce."""
                r0 = tcc * c.QCW
                yTc = ypool.tile([128, c.PAIRS * c.QCW], BF16, tag="yT")
                for p in range(c.PAIRS):
                    att_block(p, tcc, yTc[:, :])

                for tt_loc in range(c.TPC):
                    tt = tcc * c.TPC + tt_loc
                    pt = io.tile([128, c.E], F32, tag="io")
                    for eh in range(c.EH):
                        pp = ps_sps.tile([128, c.NH], F32, tag="sps")
                        for d in range(c.PAIRS):
                            nc.tensor.matmul(
                                pp[:],
                                yTc[:, d * c.QCW + tt_loc * 128:][:, :128],
                                wot[:, d * c.E + eh * c.NH:][:, :c.NH],
                                start=(d == 0), stop=(d == c.PAIRS - 1),
                            )
                        nc.vector.tensor_copy(
                            pt[:, eh * c.NH:(eh + 1) * c.NH], pp[:]
                        )
                    nc.sync.dma_start(ar1_in[tt * 128:(tt + 1) * 128, :], pt[:])

                rows = slice(r0, r0 + c.QCW)
                if c.n_cores == 1:  # timeline/profiling variant
                    nc.sync.dma_start(ar1_out[rows, :], ar1_in[rows, :])
                else:
                    nc.gpsimd.collective_compute(
                        "AllReduce", mybir.AluOpType.add,
                        replica_groups=pairs_rg,
                        ins=[ar1_in[rows, :]], outs=[ar1_out[rows, :]],
                    )

            def phase2a(tcc):
                """x2 + LN2 -> transposed h2 chunk (bf16); returns it."""
                hTc = hpool.tile([128, c.EC * c.QCW], BF16, tag="hT")

                for tt_loc in range(c.TPC):
                    tt = tcc * c.TPC + tt_loc
                    xt = io.tile([128, c.E], F32, tag="io")
                    nc.sync.dma_start(xt[:], x_d[tt * 128:(tt + 1) * 128, :])
                    at = io.tile([128, c.E], F32, tag="io")
                    nc.sync.dma_start(
                        at[:], ar1_out[tt * 128:(tt + 1) * 128, :]
                    )
                    x2 = io.tile([128, c.E], F32, tag="io")
                    eng = nc.gpsimd if tt_loc % 2 else nc.vector
                    eng.tensor_add(x2[:], xt[:], at[:])
                    eng.tensor_add(x2[:], x2[:], bo_r[:])
                    h2 = layernorm_tile(x2)
                    transpose_to(h2, hTc[:, :], tt_loc)
                return hTc

            def phase2b(tcc, hTc, halves=1):
                """FFN -> ar2_in chunk + per-half AllReduce."""
                def w2_load(eh, fg):
                    t = w2_pool.tile([128, 2 * c.NH], BF16, tag="w2")
                    nc.sync.dma_start(
                        t[:].rearrange("p (g n) -> p g n", g=2),
                        w2_d[
                            2 * fg * 128:(2 * fg + 2) * 128,
                            eh * c.NH:(eh + 1) * c.NH,
                        ].rearrange("(g p) n -> p g n", p=128),
                    )
                    return t

                # FFN layer 1: aT[f] = relu(w1_f.T @ h2T + b1_f)
                # (prefetch first ffn2 w2 tiles under the tail of this loop)
                w2_pre = {}
                nfg = c.FC // 2
                for fg in range(nfg):
                    w1t = w1_pool.tile([128, 2 * c.E], BF16, tag="w1")
                    nc.sync.dma_start(
                        w1t[:].rearrange("p (e m) -> p e m", e=c.EC),
                        w1_d[:, 2 * fg * 128:(2 * fg + 2) * 128].rearrange(
                            "(e p) m -> p e m", p=128
                        ),
                    )
                    if fg >= nfg - 2:
                        k = fg - (nfg - 2)
                        w2_pre[(0, k)] = w2_load(0, k)
                    for gi in range(2):
                        f = 2 * fg + gi
                        ap_ = ps_acc.tile([128, c.QCW], F32, tag="acc")
                        for e in range(c.EC):
                            nc.tensor.matmul(
                                ap_[:],
                                w1t[:, e * 256 + gi * 128:][:, :128],
                                hTc[:, e * c.QCW:(e + 1) * c.QCW],
                                start=(e == 0), stop=(e == c.EC - 1),
                            )
                        nc.scalar.activation(
                            aT(f), ap_[:], mybir.ActivationFunctionType.Relu,
                            bias=b1_sb[:, f:f + 1],
                        )

                # FFN layer 2: ff[t, e] = sum_f aT[f].T @ w2_f
                TPH = c.TPC // halves
                for hv in range(halves):
                    for eh in range(c.EH):
                        ffps = []
                        for _fi in range(TPH):
                            fftile = ps_sps.tile([128, c.NH], F32, tag="sps")
                            ffps.append(fftile)
                        for fg in range(nfg):
                            w2t = w2_pre.pop((eh, fg), None)
                            if w2t is None:
                                w2t = w2_load(eh, fg)
                            for gi in range(2):
                                f = 2 * fg + gi
                                for ti in range(TPH):
                                    tt_loc = hv * TPH + ti
                                    nc.tensor.matmul(
                                        ffps[ti][:],
                                        aT(f)[
                                            :,
                                            tt_loc * 128:(tt_loc + 1) * 128,
                                        ],
                                        w2t[:, gi * c.NH:(gi + 1) * c.NH],
                                        start=(f == 0), stop=(f == c.FC - 1),
                                    )
                        for ti in range(TPH):
                            tt = tcc * c.TPC + hv * TPH + ti
                            ft = io.tile([128, c.NH], F32, tag="ffout")
                            nc.vector.tensor_copy(ft[:], ffps[ti][:])
                            nc.sync.dma_start(
                                ar2_in[tt * 128:(tt + 1) * 128, eh * c.NH:][
                                    :, :c.NH
                                ],
                                ft[:],
                            )
                    r0 = tcc * c.QCW + hv * TPH * 128
                    rows = slice(r0, r0 + TPH * 128)
                    if c.n_cores == 1:
                        nc.sync.dma_start(ar2_out[rows, :], ar2_in[rows, :])
                    else:
                        nc.gpsimd.collective_compute(
                            "AllReduce", mybir.AluOpType.add,
                            replica_groups=pairs_rg,
                            ins=[ar2_in[rows, :]], outs=[ar2_out[rows, :]],
                        )

            def final(tcc):
                """out chunk = (x + attn_sum + bo) + ff_sum + b2."""
                for tt_loc in range(c.TPC):
                    tt = tcc * c.TPC + tt_loc
                    rows = slice(tt * 128, (tt + 1) * 128)
                    xt = io.tile([128, c.E], F32, tag="io")
                    nc.sync.dma_start(xt[:], x_d[rows, :])
                    at = io.tile([128, c.E], F32, tag="io")
                    nc.sync.dma_start(at[:], ar1_out[rows, :])
                    ft = io.tile([128, c.E], F32, tag="io")
                    nc.sync.dma_start(ft[:], ar2_out[rows, :])
                    ot = io.tile([128, c.E], F32, tag="io")
                    eng = nc.vector if tt_loc % 2 else nc.gpsimd
                    eng.tensor_add(ot[:], xt[:], at[:])
                    eng.tensor_add(ot[:], ot[:], ft[:])
                    eng.tensor_add(ot[:], ot[:], bob2[:])
                    nc.sync.dma_start(out_d[rows, :], ot[:])

            # ---- software-pipelined emission over chunks ----
            h2prev = None
            for tcc in range(c.TC):
                wot = phase1a(tcc)
                if tcc >= 1:
                    h2prev = phase2a(tcc - 1)
                phase1b(tcc, wot)
                if tcc >= 2:
                    final(tcc - 2)
                if tcc >= 1:
                    phase2b(tcc - 1, h2prev)
            h2prev = phase2a(c.TC - 1)
            phase2b(c.TC - 1, h2prev, halves=2)
            for tcc in range(max(0, c.TC - 2), c.TC):
                final(tcc)

    nc.compile()
    return nc


def make_masks(cfg):
    c = cfg
    m = np.zeros((128, c.KTPQ * c.QCW), dtype=np.float32)
    for j in range(c.KTPQ):
        k = np.arange(128)[:, None]
        q = np.arange(c.QCW)[None, :]
        m[:, j * c.QCW:(j + 1) * c.QCW] = (j * 128 + k <= q).astype(np.float32)
    return np.ascontiguousarray(m.astype(ml_dtypes.bfloat16))


def make_in_maps(cfg, inputs):
    """Build the per-core input dicts from the full problem inputs."""
    c = cfg
    x = np.asarray(inputs["x"], dtype=np.float32)
    ln1 = np.asarray(inputs["ln1_w"], dtype=np.float32)
    ln2 = np.asarray(inputs["ln2_w"], dtype=np.float32)
    # fold LN scale vectors into the consumer weight rows
    Wq = ln1[:, None] * np.asarray(inputs["Wq"], dtype=np.float32)
    Wk = ln1[:, None] * np.asarray(inputs["Wk"], dtype=np.float32)
    Wv = ln1[:, None] * np.asarray(inputs["Wv"], dtype=np.float32)
    W1 = ln2[:, None] * np.asarray(inputs["W1"], dtype=np.float32)
    Wo = np.asarray(inputs["Wo"], dtype=np.float32)
    W2 = np.asarray(inputs["W2"], dtype=np.float32)
    bo = np.asarray(inputs["bo"], dtype=np.float32)
    b1 = np.asarray(inputs["b1"], dtype=np.float32)
    b2 = np.asarray(inputs["b2"], dtype=np.float32)

    def rep(v):
        return np.ascontiguousarray(
            np.broadcast_to(v[None, :], (128, c.E)).astype(np.float32)
        )

    consts = {
        "bor": rep(bo), "b2r": rep(bo + b2),
        "masks": make_masks(c),
        "ident": np.eye(128, dtype=np.float32),
    }
    in_maps = []
    for core in range(c.n_cores):
        b, g = core // 2, core % 2
        d0, d1 = g * c.D, (g + 1) * c.D
        f0, f1 = g * c.FH, (g + 1) * c.FH
        m = {
            "x": np.ascontiguousarray(x[b]),
            "wq": np.ascontiguousarray(Wq[:, d0:d1].astype(ml_dtypes.bfloat16)),
            "wk": np.ascontiguousarray(Wk[:, d0:d1].astype(ml_dtypes.bfloat16)),
            "wv": np.ascontiguousarray(Wv[:, d0:d1].astype(ml_dtypes.bfloat16)),
            "wo": np.ascontiguousarray(Wo[d0:d1, :].astype(ml_dtypes.bfloat16)),
            "w1": np.ascontiguousarray(W1[:, f0:f1].astype(ml_dtypes.bfloat16)),
            "w2": np.ascontiguousarray(W2[f0:f1, :].astype(ml_dtypes.bfloat16)),
            "b1": np.ascontiguousarray(b1[f0:f1].reshape(c.FC, 128).T),
        }
        m.update(consts)
        in_maps.append(m)
    return in_maps


_NC_CACHE = {}


def get_nc(cfg):
    key = (cfg.B, cfg.T, cfg.E, cfg.HPC, cfg.FH, cfg.n_cores)
    if key not in _NC_CACHE:
        _NC_CACHE[key] = build_nc(cfg)
    return _NC_CACHE[key]


def kernel(**inputs) -> np.ndarray:
    c = FULL
    nc = get_nc(c)
    in_maps = make_in_maps(c, inputs)
    res = run_bass_kernel_spmd(nc, in_maps, core_ids=list(range(c.n_cores)))
    out = np.stack([res.results[2 * b]["out"] for b in range(c.B)], axis=0)
    return out.astype(np.float32)


# revision 24
# speedup vs baseline: 2203.8199x; 61.1439x over previous
"""Trainium2 Bass kernel for a full transformer block (LN->MHA->LN->FFN).

Sharding: 4-way data-parallel over batch x 2-way tensor-parallel
(heads + FFN hidden split), pairwise AllReduce after attention-proj and FFN.
Core c handles batch c//2 with head-group / hidden-slice c%2.

The program is software-pipelined over t-chunks of 512 rows: phase1 =
LN1+QKV+attention+proj -> per-chunk AllReduce; phase2 = x2+LN2+FFN ->
per-chunk AllReduce -> residual out. Phase2(tc-1) is emitted after
phase1(tc) so collective latency hides under compute (engines run their
instruction streams in order).

LayerNorm scale vectors are folded into Wq/Wk/Wv/W1 rows on the host.

Self-contained: hardcodes the full-problem shapes; builds per-core input
slices on the host, runs one SPMD Bass program on 8 NeuronCores.
"""

import numpy as np
import ml_dtypes

import concourse.bacc as bacc
import concourse.tile as tile
from concourse import mybir
from concourse.bass_utils import run_bass_kernel_spmd

F32 = mybir.dt.float32
F32R = mybir.dt.float32r
BF16 = mybir.dt.bfloat16
EPS = 1e-5


class Cfg:
    def __init__(self, B, T, E, HPC, FH, n_cores):
        self.B, self.T, self.E, self.HPC, self.FH = B, T, E, HPC, FH
        self.n_cores = n_cores
        self.HS = 64
        self.D = HPC * self.HS          # local head dims (= cols of Wq slice)
        self.PAIRS = HPC // 2           # 128-col head-pair groups
        self.TT = T // 128              # t-tiles
        self.QCW = min(512, T)          # q-chunk width for attention
        self.TC = T // self.QCW        # q/t-chunks
        self.KTPQ = self.QCW // 128     # k-tiles per q-chunk block
        self.TPC = self.QCW // 128      # t-tiles per chunk
        self.EC = E // 128              # e-chunks
        self.NH = min(512, E)           # matmul N for E-wide outputs
        self.EH = E // self.NH          # n-halves of E
        self.FC = FH // 128             # FFN hidden chunks
        self.scale = 1.0 / np.sqrt(E)


FULL = Cfg(B=4, T=2048, E=1024, HPC=8, FH=2048, n_cores=8)


def build_nc(cfg):
    c = cfg
    nc = bacc.Bacc(
        "TRN2", target_bir_lowering=False, debug=False, num_devices=c.n_cores
    )
    pairs_rg = [[2 * i, 2 * i + 1] for i in range(c.n_cores // 2)]

    # ---- DRAM I/O ----
    x_d = nc.dram_tensor("x", [c.T, c.E], F32, kind="ExternalInput")
    wq_d = nc.dram_tensor("wq", [c.E, c.D], BF16, kind="ExternalInput")
    wk_d = nc.dram_tensor("wk", [c.E, c.D], BF16, kind="ExternalInput")
    wv_d = nc.dram_tensor("wv", [c.E, c.D], BF16, kind="ExternalInput")
    wo_d = nc.dram_tensor("wo", [c.D, c.E], BF16, kind="ExternalInput")
    w1_d = nc.dram_tensor("w1", [c.E, c.FH], BF16, kind="ExternalInput")
    w2_d = nc.dram_tensor("w2", [c.FH, c.E], BF16, kind="ExternalInput")
    b1_d = nc.dram_tensor("b1", [128, c.FC], F32, kind="ExternalInput")
    bo_d = nc.dram_tensor("bor", [128, c.E], F32, kind="ExternalInput")
    b2_d = nc.dram_tensor("b2r", [128, c.E], F32, kind="ExternalInput")
    msk_d = nc.dram_tensor(
        "masks", [128, c.KTPQ * c.QCW], BF16, kind="ExternalInput"
    )
    id_d = nc.dram_tensor("ident", [128, 128], F32R, kind="ExternalInput")
    out_d = nc.dram_tensor("out", [c.T, c.E], F32, kind="ExternalOutput")

    # ---- persistent SBUF ----
    qkT = nc.alloc_sbuf_tensor("qkT", [128, 2 * c.PAIRS * c.T], BF16).ap()

    def qT(p):
        return qkT[:, p * c.T:(p + 1) * c.T]

    def kT(p):
        return qkT[:, (c.PAIRS + p) * c.T:(c.PAIRS + p + 1) * c.T]

    aT_sb = nc.alloc_sbuf_tensor("aT_sb", [128, c.FC * c.QCW], BF16).ap()

    def aT(f):
        return aT_sb[:, f * c.QCW:(f + 1) * c.QCW]

    v_sb = nc.alloc_sbuf_tensor("v_sb", [128, c.TT * c.HPC * 65], BF16).ap()

    def v_aug(tt, h):
        o = (tt * c.HPC + h) * 65
        return v_sb[:, o:o + 65]

    NSLOT = 6
    attT = nc.alloc_sbuf_tensor("attT", [128, NSLOT * c.QCW], BF16).ap()

    ident = nc.alloc_sbuf_tensor("ident_sb", [128, 128], F32R).ap()
    masks = nc.alloc_sbuf_tensor("masks_sb", [128, c.KTPQ * c.QCW], BF16).ap()
    bo_r = nc.alloc_sbuf_tensor("bo_sb", [128, c.E], F32).ap()
    bob2 = nc.alloc_sbuf_tensor("bob2_sb", [128, c.E], F32).ap()
    b1_sb = nc.alloc_sbuf_tensor("b1_sb", [128, c.FC], F32).ap()

    # ---- internal DRAM ----
    ar1_in = nc.dram_tensor("ar1_in", [c.T, c.E], F32, kind="Internal")
    ar1_out = nc.dram_tensor("ar1_out", [c.T, c.E], F32, kind="Internal")
    ar2_in = nc.dram_tensor("ar2_in", [c.T, c.E], F32, kind="Internal")
    ar2_out = nc.dram_tensor("ar2_out", [c.T, c.E], F32, kind="Internal")

    with tile.TileContext(nc) as tc:
        with (
            tc.tile_pool(name="io", bufs=4) as io,
            tc.tile_pool(name="hT", bufs=2) as hpool,
            tc.tile_pool(name="yTp", bufs=2) as ypool,
            tc.tile_pool(name="scr", bufs=2) as scr,
            tc.tile_pool(name="stat", bufs=2) as stat,
            tc.tile_pool(name="wqk", bufs=3) as wqk_pool,
            tc.tile_pool(name="w1p", bufs=2) as w1_pool,
            tc.tile_pool(name="w2p", bufs=3) as w2_pool,
            tc.tile_pool(name="wvp", bufs=1) as wv_pool,
            tc.tile_pool(name="wop", bufs=1) as wo_pool,
            tc.tile_pool(name="rcp", bufs=1) as rcp,
            tc.tile_pool(name="ps_sps", bufs=4, space="PSUM") as ps_sps,
            tc.tile_pool(name="ps_tp", bufs=1, space="PSUM") as ps_tp,
            tc.tile_pool(name="ps_acc", bufs=1, space="PSUM") as ps_acc,
            tc.tile_pool(name="ps_yps", bufs=1, space="PSUM") as ps_yps,
        ):
            # ---- consts ----
            nc.gpsimd.dma_start(ident[:], id_d[:])
            nc.gpsimd.dma_start(masks[:], msk_d[:])
            nc.gpsimd.dma_start(bo_r[:], bo_d[:])
            nc.gpsimd.dma_start(bob2[:], b2_d[:])
            nc.gpsimd.dma_start(b1_sb[:], b1_d[:])

            def layernorm_tile(xt):
                """xt: [128, E] f32 SBUF -> h [128, E] f32r tile.

                Scale weight is pre-folded into the consumer matmul weights.
                rsqrt(v) = exp(-0.5*ln(v)) keeps ACT on one table set
                (natural_log_exp_and_others: exp+ln+relu) for the kernel.
                """
                ng = c.E // 512
                bst = stat.tile([128, 6 * ng], F32, tag="bst")
                bst3 = bst[:].rearrange("p (g s) -> p g s", g=ng)
                for g in range(ng):
                    nc.vector.bn_stats(
                        bst3[:, g:g + 1, :],
                        xt[:, g * 512:(g + 1) * 512].rearrange(
                            "p (g w) -> p g w", g=1
                        ),
                    )
                mv = stat.tile([128, 2], F32, tag="mv")
                nc.vector.bn_aggr(
                    mv[:], bst[:].rearrange("p (g s) -> p g s", g=ng)
                )
                mu = mv[:, 0:1]
                varp = stat.tile([128, 1], F32, tag="varp")
                nc.vector.tensor_scalar_add(varp[:], mv[:, 1:2], EPS)
                lnv = stat.tile([128, 1], F32, tag="lnv")
                nc.scalar.activation(
                    lnv[:], varp[:], mybir.ActivationFunctionType.Ln
                )
                rsig = stat.tile([128, 1], F32, tag="rsig")
                nc.scalar.activation(
                    rsig[:], lnv[:], mybir.ActivationFunctionType.Exp,
                    scale=-0.5,
                )
                h = scr.tile([128, c.E], F32R, tag="h")
                nc.vector.tensor_scalar(
                    h[:], xt[:], mu, rsig[:],
                    mybir.AluOpType.subtract, mybir.AluOpType.mult,
                )
                return h

            TG = 4  # transposes per psum tile

            def transpose_to(h, hTc, tt_loc):
                """h [128,E] f32r -> hTc e-chunk columns tt_loc (transposed)."""
                dst3 = hTc.rearrange("p (e w) -> p e w", e=c.EC)[
                    :, :, tt_loc * 128:(tt_loc + 1) * 128
                ]
                for g0 in range(0, c.EC, TG):
                    tp = ps_tp.tile([128, TG * 128], F32R, tag="tp")
                    for i in range(TG):
                        e = g0 + i
                        nc.tensor.matmul(
                            tp[:, i * 128:(i + 1) * 128],
                            h[:, e * 128:(e + 1) * 128],
                            ident[:],
                            is_transpose=True, start=True, stop=True,
                        )
                    nc.vector.tensor_copy(
                        dst3[:, g0:g0 + TG, :],
                        tp[:].rearrange("p (g w) -> p g w", g=TG),
                    )

            slot_ctr = [0]

            def att_block(p, qc, yTc):
                """Attention for head pair p, q-chunk qc (kT/v ready)."""
                last = c.KTPQ * qc + c.KTPQ - 1
                q0 = qc * c.QCW
                yps_e = ps_yps.tile([65, c.QCW], F32, tag="yps_e")
                yps_o = ps_yps.tile([65, c.QCW], F32, tag="yps_o")
                pend = []

                def issue_av(kt, cq0, a_e, a_o):
                    st, sp = kt == 0, kt == last
                    nc.tensor.matmul(
                        yps_e[:, cq0:], v_aug(kt, 2 * p), a_e[:, cq0:],
                        start=st, stop=sp,
                    )
                    nc.tensor.matmul(
                        yps_o[:, cq0:], v_aug(kt, 2 * p + 1), a_o[:, cq0:],
                        start=st, stop=sp,
                    )

                for kt in range(last + 1):
                    j = kt - c.KTPQ * qc  # >=0: diagonal block stripe
                    # columns q < j*128 are fully masked: skip them entirely
                    cq0 = max(0, j) * 128
                    ncols = c.QCW - cq0
                    sps_e = ps_sps.tile([128, c.QCW], F32, tag="sps")
                    sps_o = ps_sps.tile([128, c.QCW], F32, tag="sps")
                    for hh, sps in ((0, sps_e), (1, sps_o)):
                        off = hh * 64
                        nc.tensor.matmul(
                            sps[:, cq0:],
                            kT(p)[off:off + 64, kt * 128:(kt + 1) * 128],
                            qT(p)[off:off + 64, q0 + cq0:q0 + c.QCW],
                            start=True, stop=True,
                            tile_position=(off, 0),
                        )
                    s0 = (slot_ctr[0] % NSLOT) * c.QCW
                    a_e = attT[:, s0:s0 + c.QCW]
                    slot_ctr[0] += 1
                    s1 = (slot_ctr[0] % NSLOT) * c.QCW
                    a_o = attT[:, s1:s1 + c.QCW]
                    slot_ctr[0] += 1
                    nc.scalar.activation(
                        a_e[:, cq0:], sps_e[:, cq0:],
                        mybir.ActivationFunctionType.Exp,
                    )
                    nc.scalar.activation(
                        a_o[:, cq0:], sps_o[:, cq0:],
                        mybir.ActivationFunctionType.Exp,
                    )
                    if j >= 0:  # triangular mask on the surviving stripe
                        m = masks[:, j * c.QCW + cq0:(j + 1) * c.QCW]
                        nc.vector.tensor_mul(a_e[:, cq0:], a_e[:, cq0:], m)
                        nc.vector.tensor_mul(a_o[:, cq0:], a_o[:, cq0:], m)
                    pend.append((kt, cq0, a_e, a_o))
                    if len(pend) > 1:
                        issue_av(*pend.pop(0))
                while pend:
                    issue_av(*pend.pop(0))

                # normalize: yTc[p] rows = yps[0:64] * (1/yps[64])
                for hh, yps in ((0, yps_e), (1, yps_o)):
                    rc = rcp.tile([1, c.QCW], F32, tag="rc")
                    nc.vector.reciprocal(rc[:], yps[64:65, :])
                    rb = rcp.tile([64, c.QCW], F32, tag="rb")
                    nc.gpsimd.partition_broadcast(rb[:], rc[:])
                    nc.vector.tensor_mul(
                        yTc[hh * 64:hh * 64 + 64, p * c.QCW:(p + 1) * c.QCW],
                        rb[:], yps[0:64, :],
                    )

            def phase1a(tcc):
                """LN1 + V + QK for chunk tcc; returns wot for phase1b."""
                r0 = tcc * c.QCW  # first row of chunk
                hTc = hpool.tile([128, c.EC * c.QCW], BF16, tag="hT")

                # x tiles first so LN can start before weight DMAs queue
                xts = []
                for tt_loc in range(c.TPC):
                    tt = tcc * c.TPC + tt_loc
                    xt = io.tile([128, c.E], F32, tag="io")
                    nc.sync.dma_start(xt[:], x_d[tt * 128:(tt + 1) * 128, :])
                    xts.append(xt)

                # weight prefetch for this chunk
                wvt = wv_pool.tile([128, c.EC * c.D], BF16, tag="wv")
                nc.sync.dma_start(
                    wvt[:].rearrange("p (e d) -> p e d", e=c.EC),
                    wv_d[:].rearrange("(e p) d -> p e d", p=128),
                )
                wot = wo_pool.tile([128, c.PAIRS * c.E], BF16, tag="wo")
                nc.sync.dma_start(
                    wot[:].rearrange("p (d e) -> p d e", d=c.PAIRS),
                    wo_d[:].rearrange("(d p) e -> p d e", p=128),
                )

                for tt_loc in range(c.TPC):
                    h = layernorm_tile(xts[tt_loc])
                    transpose_to(h, hTc[:, :], tt_loc)

                # V for this chunk's t-tiles
                for tt_loc in range(c.TPC):
                    tt = tcc * c.TPC + tt_loc
                    vps = ps_acc.tile([128, c.D], F32, tag="acc")
                    for e in range(c.EC):
                        nc.tensor.matmul(
                            vps[:],
                            hTc[:, e * c.QCW + tt_loc * 128:][:, :128],
                            wvt[:, e * c.D:(e + 1) * c.D],
                            start=(e == 0), stop=(e == c.EC - 1),
                        )
                    vdst = v_sb[
                        :, tt * c.HPC * 65:(tt + 1) * c.HPC * 65
                    ].rearrange("p (h w) -> p h w", w=65)
                    nc.vector.tensor_copy(
                        vdst[:, :, 0:64],
                        vps[:].rearrange("p (h w) -> p h w", w=64),
                    )
                    nc.vector.memset(vdst[:, :, 64:65], 1.0)

                # Q/K for this chunk
                for p in range(c.PAIRS):
                    wqt = wqk_pool.tile([128, c.E], BF16, tag="wqk")
                    nc.sync.dma_start(
                        wqt[:].rearrange("p (e m) -> p e m", e=c.EC),
                        wq_d[:, p * 128:(p + 1) * 128].rearrange(
                            "(e p) m -> p e m", p=128
                        ),
                    )
                    wkt = wqk_pool.tile([128, c.E], BF16, tag="wqk")
                    nc.sync.dma_start(
                        wkt[:].rearrange("p (e m) -> p e m", e=c.EC),
                        wk_d[:, p * 128:(p + 1) * 128].rearrange(
                            "(e p) m -> p e m", p=128
                        ),
                    )
                    for dst, wt, do_scale in (
                        (qT, wqt, True), (kT, wkt, False)
                    ):
                        pp = ps_sps.tile([128, c.QCW], F32, tag="sps")
                        for e in range(c.EC):
                            nc.tensor.matmul(
                                pp[:], wt[:, e * 128:(e + 1) * 128],
                                hTc[:, e * c.QCW:(e + 1) * c.QCW],
                                start=(e == 0), stop=(e == c.EC - 1),
                            )
                        d = dst(p)[:, r0:r0 + c.QCW]
                        if do_scale:
                            nc.vector.tensor_scalar_mul(d, pp[:], c.scale)
                        else:
                            nc.vector.tensor_copy(d, pp[:])

                return wot

            def phase1b(tcc, wot):
                """Attention + proj -> ar1_in chunk + AllReduce."""
                r0 = tcc * c.QCW
                yTc = ypool.tile([128, c.PAIRS * c.QCW], BF16, tag="yT")
                for p in range(c.PAIRS):
                    att_block(p, tcc, yTc[:, :])

                for tt_loc in range(c.TPC):
                    tt = tcc * c.TPC + tt_loc
                    pt = io.tile([128, c.E], F32, tag="io")
                    for eh in range(c.EH):
                        pp = ps_sps.tile([128, c.NH], F32, tag="sps")
                        for d in range(c.PAIRS):
                            nc.tensor.matmul(
                                pp[:],
                                yTc[:, d * c.QCW + tt_loc * 128:][:, :128],
                                wot[:, d * c.E + eh * c.NH:][:, :c.NH],
                                start=(d == 0), stop=(d == c.PAIRS - 1),
                            )
                        nc.vector.tensor_copy(
                            pt[:, eh * c.NH:(eh + 1) * c.NH], pp[:]
                        )
                    nc.sync.dma_start(ar1_in[tt * 128:(tt + 1) * 128, :], pt[:])

                rows = slice(r0, r0 + c.QCW)
                if c.n_cores == 1:  # timeline/profiling variant
                    nc.sync.dma_start(ar1_out[rows, :], ar1_in[rows, :])
                else:
                    nc.gpsimd.collective_compute(
                        "AllReduce", mybir.AluOpType.add,
                        replica_groups=pairs_rg,
                        ins=[ar1_in[rows, :]], outs=[ar1_out[rows, :]],
                    )

            def phase2a(tcc):
                """x2 + LN2 -> transposed h2 chunk (bf16); returns it."""
                hTc = hpool.tile([128, c.EC * c.QCW], BF16, tag="hT")

                for tt_loc in range(c.TPC):
                    tt = tcc * c.TPC + tt_loc
                    xt = io.tile([128, c.E], F32, tag="io")
                    nc.sync.dma_start(xt[:], x_d[tt * 128:(tt + 1) * 128, :])
                    at = io.tile([128, c.E], F32, tag="io")
                    nc.sync.dma_start(
                        at[:], ar1_out[tt * 128:(tt + 1) * 128, :]
                    )
                    x2 = io.tile([128, c.E], F32, tag="io")
                    eng = nc.gpsimd if tt_loc % 2 else nc.vector
                    eng.tensor_add(x2[:], xt[:], at[:])
                    eng.tensor_add(x2[:], x2[:], bo_r[:])
                    h2 = layernorm_tile(x2)
                    transpose_to(h2, hTc[:, :], tt_loc)
                return hTc

            def w1_load(fg):
                w1t = w1_pool.tile([128, 2 * c.E], BF16, tag="w1")
                nc.sync.dma_start(
                    w1t[:].rearrange("p (e m) -> p e m", e=c.EC),
                    w1_d[:, 2 * fg * 128:(2 * fg + 2) * 128].rearrange(
                        "(e p) m -> p e m", p=128
                    ),
                )
                return w1t

            def phase2b(tcc, hTc, halves=1, w1pre=None):
                """FFN -> ar2_in chunk + per-half AllReduce."""
                def w2_load(eh, fg):
                    t = w2_pool.tile([128, 2 * c.NH], BF16, tag="w2")
                    nc.scalar.dma_start(
                        t[:].rearrange("p (g n) -> p g n", g=2),
                        w2_d[
                            2 * fg * 128:(2 * fg + 2) * 128,
                            eh * c.NH:(eh + 1) * c.NH,
                        ].rearrange("(g p) n -> p g n", p=128),
                    )
                    return t

                # FFN layer 1: aT[f] = relu(w1_f.T @ h2T + b1_f)
                # (prefetch first ffn2 w2 tiles under the tail of this loop)
                w2_pre = {}
                nfg = c.FC // 2
                for fg in range(nfg):
                    if w1pre is not None and fg == 0:
                        w1t = w1pre
                    else:
                        w1t = w1_load(fg)
                    if fg >= nfg - 2:
                        k = fg - (nfg - 2)
                        w2_pre[(0, k)] = w2_load(0, k)
                    for gi in range(2):
                        f = 2 * fg + gi
                        ap_ = ps_acc.tile([128, c.QCW], F32, tag="acc")
                        for e in range(c.EC):
                            nc.tensor.matmul(
                                ap_[:],
                                w1t[:, e * 256 + gi * 128:][:, :128],
                                hTc[:, e * c.QCW:(e + 1) * c.QCW],
                                start=(e == 0), stop=(e == c.EC - 1),
                            )
                        nc.scalar.activation(
                            aT(f), ap_[:], mybir.ActivationFunctionType.Relu,
                            bias=b1_sb[:, f:f + 1],
                        )

                # FFN layer 2: ff[t, e] = sum_f aT[f].T @ w2_f
                TPH = c.TPC // halves
                for hv in range(halves):
                    for eh in range(c.EH):
                        ffps = []
                        for _fi in range(TPH):
                            fftile = ps_sps.tile([128, c.NH], F32, tag="sps")
                            ffps.append(fftile)
                        for fg in range(nfg):
                            w2t = w2_pre.pop((eh, fg), None)
                            if w2t is None:
                                w2t = w2_load(eh, fg)
                            for gi in range(2):
                                f = 2 * fg + gi
                                for ti in range(TPH):
                                    tt_loc = hv * TPH + ti
                                    nc.tensor.matmul(
                                        ffps[ti][:],
                                        aT(f)[
                                            :,
                                            tt_loc * 128:(tt_loc + 1) * 128,
                                        ],
                                        w2t[:, gi * c.NH:(gi + 1) * c.NH],
                                        start=(f == 0), stop=(f == c.FC - 1),
                                    )
                        for ti in range(TPH):
                            tt = tcc * c.TPC + hv * TPH + ti
                            ft = io.tile([128, c.NH], F32, tag="ffout")
                            nc.vector.tensor_copy(ft[:], ffps[ti][:])
                            nc.sync.dma_start(
                                ar2_in[tt * 128:(tt + 1) * 128, eh * c.NH:][
                                    :, :c.NH
                                ],
                                ft[:],
                            )
                    r0 = tcc * c.QCW + hv * TPH * 128
                    rows = slice(r0, r0 + TPH * 128)
                    if c.n_cores == 1:
                        nc.sync.dma_start(ar2_out[rows, :], ar2_in[rows, :])
                    else:
                        nc.gpsimd.collective_compute(
                            "AllReduce", mybir.AluOpType.add,
                            replica_groups=pairs_rg,
                            ins=[ar2_in[rows, :]], outs=[ar2_out[rows, :]],
                        )

            def final(tcc):
                """out chunk = (x + attn_sum + bo) + ff_sum + b2."""
                for tt_loc in range(c.TPC):
                    tt = tcc * c.TPC + tt_loc
                    rows = slice(tt * 128, (tt + 1) * 128)
                    xt = io.tile([128, c.E], F32, tag="io")
                    nc.scalar.dma_start(xt[:], x_d[rows, :])
                    at = io.tile([128, c.E], F32, tag="io")
                    nc.scalar.dma_start(at[:], ar1_out[rows, :])
                    ft = io.tile([128, c.E], F32, tag="io")
                    nc.scalar.dma_start(ft[:], ar2_out[rows, :])
                    ot = io.tile([128, c.E], F32, tag="io")
                    eng = nc.vector if tt_loc % 2 else nc.gpsimd
                    eng.tensor_add(ot[:], xt[:], at[:])
                    eng.tensor_add(ot[:], ot[:], ft[:])
                    eng.tensor_add(ot[:], ot[:], bob2[:])
                    nc.sync.dma_start(out_d[rows, :], ot[:])

            # ---- software-pipelined emission over chunks ----
            h2prev = None
            for tcc in range(c.TC):
                wot = phase1a(tcc)
                w1pre = w1_load(0) if tcc >= 1 else None
                if tcc >= 1:
                    h2prev = phase2a(tcc - 1)
                phase1b(tcc, wot)
                if tcc >= 2:
                    final(tcc - 2)
                if tcc >= 1:
                    phase2b(tcc - 1, h2prev, w1pre=w1pre)
            w1pre = w1_load(0)
            h2prev = phase2a(c.TC - 1)
            phase2b(c.TC - 1, h2prev, halves=2, w1pre=w1pre)
            for tcc in range(max(0, c.TC - 2), c.TC):
                final(tcc)

    nc.compile()
    return nc


def make_masks(cfg):
    c = cfg
    m = np.zeros((128, c.KTPQ * c.QCW), dtype=np.float32)
    for j in range(c.KTPQ):
        k = np.arange(128)[:, None]
        q = np.arange(c.QCW)[None, :]
        m[:, j * c.QCW:(j + 1) * c.QCW] = (j * 128 + k <= q).astype(np.float32)
    return np.ascontiguousarray(m.astype(ml_dtypes.bfloat16))


def make_in_maps(cfg, inputs):
    """Build the per-core input dicts from the full problem inputs."""
    c = cfg
    x = np.asarray(inputs["x"], dtype=np.float32)
    ln1 = np.asarray(inputs["ln1_w"], dtype=np.float32)
    ln2 = np.asarray(inputs["ln2_w"], dtype=np.float32)
    # fold LN scale vectors into the consumer weight rows
    Wq = ln1[:, None] * np.asarray(inputs["Wq"], dtype=np.float32)
    Wk = ln1[:, None] * np.asarray(inputs["Wk"], dtype=np.float32)
    Wv = ln1[:, None] * np.asarray(inputs["Wv"], dtype=np.float32)
    W1 = ln2[:, None] * np.asarray(inputs["W1"], dtype=np.float32)
    Wo = np.asarray(inputs["Wo"], dtype=np.float32)
    W2 = np.asarray(inputs["W2"], dtype=np.float32)
    bo = np.asarray(inputs["bo"], dtype=np.float32)
    b1 = np.asarray(inputs["b1"], dtype=np.float32)
    b2 = np.asarray(inputs["b2"], dtype=np.float32)

    def rep(v):
        return np.ascontiguousarray(
            np.broadcast_to(v[None, :], (128, c.E)).astype(np.float32)
        )

    consts = {
        "bor": rep(bo), "b2r": rep(bo + b2),
        "masks": make_masks(c),
        "ident": np.eye(128, dtype=np.float32),
    }
    in_maps = []
    for core in range(c.n_cores):
        b, g = core // 2, core % 2
        d0, d1 = g * c.D, (g + 1) * c.D
        f0, f1 = g * c.FH, (g + 1) * c.FH
        m = {
            "x": np.ascontiguousarray(x[b]),
            "wq": np.ascontiguousarray(Wq[:, d0:d1].astype(ml_dtypes.bfloat16)),
            "wk": np.ascontiguousarray(Wk[:, d0:d1].astype(ml_dtypes.bfloat16)),
            "wv": np.ascontiguousarray(Wv[:, d0:d1].astype(ml_dtypes.bfloat16)),
            "wo": np.ascontiguousarray(Wo[d0:d1, :].astype(ml_dtypes.bfloat16)),
            "w1": np.ascontiguousarray(W1[:, f0:f1].astype(ml_dtypes.bfloat16)),
            "w2": np.ascontiguousarray(W2[f0:f1, :].astype(ml_dtypes.bfloat16)),
            "b1": np.ascontiguousarray(b1[f0:f1].reshape(c.FC, 128).T),
        }
        m.update(consts)
        in_maps.append(m)
    return in_maps


_NC_CACHE = {}


def get_nc(cfg):
    key = (cfg.B, cfg.T, cfg.E, cfg.HPC, cfg.FH, cfg.n_cores)
    if key not in _NC_CACHE:
        _NC_CACHE[key] = build_nc(cfg)
    return _NC_CACHE[key]


def kernel(**inputs) -> np.ndarray:
    c = FULL
    nc = get_nc(c)
    in_maps = make_in_maps(c, inputs)
    res = run_bass_kernel_spmd(nc, in_maps, core_ids=list(range(c.n_cores)))
    out = np.stack([res.results[2 * b]["out"] for b in range(c.B)], axis=0)
    return out.astype(np.float32)
